# revision 6
# baseline (speedup 1.0000x reference)
"""DNPUConv2d Trainium2 kernel (8 NeuronCores, batch-parallel).

Restructure of the reference computation:
  - The per-device electrode permutation is folded into W1 by row
    permutation: z1 = u_d @ A_d + controls @ C_d.
  - Control contribution cb[o,i,d,:] = all_controls[o,i,d] @ C_d + b1 is
    precomputed on host and folded into the L1 matmul as extra ones-rows
    of the rhs (zero-interleaved, 4 combos per tile, K=16).
  - ELU via the exact identity elu(z) = max(min(exp(z),1), z+1) - 1.
    The "-1" is folded into the next layer's bias; biases ride as an
    extra ones-row through the matmuls, so on device each layer is
       z' = matmul (bias+1 rides on ones-row);  e = Exp(z'-1) [ScalarE]
       g = (e min 1) max z'                     [VectorE stt]
  - The sum over (in_ch, device) is moved before the W3 dot (linearity):
    the 12 W3 matmuls per (b,o) run back-to-back (one weight load),
    PSUM-accumulating into one [1, 512] tile; host folds the halves.

Tiles are [91, 1024] (4 combos x 256 positions); per (b,o) there are 6.
Sharding: batch 16 -> 2 per core across 8 cores; weights replicated.
"""
import numpy as np

K = 3
PAD = 1
IN_CH = 8
OUT_CH = 16
DEV = 3
N_IN = 3
N_CTRL = 4
HID = 90
B = 16
HW = 16
L = HW * HW            # 256 output positions
N_CORES = 8
B_LOC = B // N_CORES   # 2 batches per core
M = HID + 1            # 91: hidden + ones row
NJ = IN_CH * DEV       # 24 (i,d) combos per o
NT = NJ // 4           # 6 tiles per (b,o), four combos each
X = 1024               # tile free dim
XH = X // 2            # matmul N (fp32 moving-operand limit)

_COMPILED = {}


def _build_program():
    import concourse.bacc as bacc
    import concourse.tile as tile
    from concourse import mybir

    f32 = mybir.dt.float32
    f32r = mybir.dt.float32r
    Exp = mybir.ActivationFunctionType.Exp
    amin = mybir.AluOpType.min
    amax = mybir.AluOpType.max

    RCOLS = B_LOC * NT * X        # 12288 zero-interleaved rhs columns

    nc = bacc.Bacc()
    rhs1_d = nc.dram_tensor("rhs1", [16, RCOLS], f32r, kind="ExternalInput")
    lhs1_d = nc.dram_tensor("lhs1", [16, OUT_CH * NT, M], f32r,
                            kind="ExternalInput")
    w2g_d = nc.dram_tensor("w2g", [M, M], f32r, kind="ExternalInput")
    w3g_d = nc.dram_tensor("w3g", [M, 1], f32r, kind="ExternalInput")
    out_d = nc.dram_tensor("out", [B_LOC * OUT_CH, XH], f32,
                           kind="ExternalOutput")

    with tile.TileContext(nc) as tc:
        with (
            tc.tile_pool(name="singles", bufs=1) as singles,
            tc.tile_pool(name="work", bufs=3) as work,
            tc.tile_pool(name="g2p", bufs=NT + 2) as g2p,
            tc.tile_pool(name="outp", bufs=2) as outp,
            tc.tile_pool(name="psz", bufs=3, space="PSUM") as psz,
            tc.tile_pool(name="psacc", bufs=2, space="PSUM") as psacc,
        ):
            rhs_sb = singles.tile([16, RCOLS], f32r)
            w2g_sb = singles.tile([M, M], f32r)
            w3g_sb = singles.tile([M, 1], f32r)
            neg1 = singles.tile([128, 1], f32)
            nc.vector.memset(neg1, -1.0)
            nc.sync.dma_start(out=rhs_sb, in_=rhs1_d[:, :])
            nc.sync.dma_start(out=w2g_sb, in_=w2g_d[:, :])
            nc.sync.dma_start(out=w3g_sb, in_=w3g_d[:, :])

            for o in range(OUT_CH):
                lhs_o = outp.tile([16, NT, M], f32r, tag="lhs_o", name="lhs_o")
                nc.sync.dma_start(out=lhs_o,
                                  in_=lhs1_d[:, o * NT:(o + 1) * NT, :])
                for b in range(B_LOC):
                    g2s = []
                    for t in range(NT):
                        col = (b * NT + t) * X
                        z1 = psz.tile([M, X], f32, tag="z", name="z1")
                        for h in range(2):
                            nc.tensor.matmul(
                                z1[:, h * XH:(h + 1) * XH],
                                lhs_o[:, t, :],
                                rhs_sb[:, col + h * XH:col + (h + 1) * XH],
                                start=True, stop=True)
                        e1 = work.tile([M, X], f32, tag="e1")
                        nc.scalar.activation(e1, z1, Exp, bias=neg1[:M],
                                             scale=1.0)
                        g1 = work.tile([M, X], f32r, tag="g1")
                        nc.vector.scalar_tensor_tensor(
                            out=g1, in0=e1, scalar=1.0, in1=z1,
                            op0=amin, op1=amax)
                        z2 = psz.tile([M, X], f32, tag="z", name="z2")
                        for h in range(2):
                            nc.tensor.matmul(
                                z2[:, h * XH:(h + 1) * XH], w2g_sb,
                                g1[:, h * XH:(h + 1) * XH],
                                start=True, stop=True)
                        e2 = work.tile([M, X], f32, tag="e2")
                        nc.scalar.activation(e2, z2, Exp, bias=neg1[:M],
                                             scale=1.0)
                        g2 = g2p.tile([M, X], f32r, tag="g2", name="g2")
                        nc.vector.scalar_tensor_tensor(
                            out=g2, in0=e2, scalar=1.0, in1=z2,
                            op0=amin, op1=amax)
                        g2s.append(g2)
                    acc = psacc.tile([1, XH], f32)
                    n3 = 0
                    for t in range(NT):
                        for h in range(2):
                            nc.tensor.matmul(
                                acc, w3g_sb, g2s[t][:, h * XH:(h + 1) * XH],
                                start=(n3 == 0), stop=(n3 == 2 * NT - 1))
                            n3 += 1
                    bo = b * OUT_CH + o
                    out_sb = outp.tile([1, XH], f32, tag="osb", name="out_sb")
                    nc.scalar.copy(out_sb, acc)
                    nc.sync.dma_start(out=out_d[bo:bo + 1, :], in_=out_sb)

    nc.compile()
    return nc


def _get_program():
    if "nc" not in _COMPILED:
        _COMPILED["nc"] = _build_program()
    return _COMPILED["nc"]


def _host_prep(x, all_controls, W1, b1, W2, b2, W3, b3,
               input_indices, control_indices):
    """Build per-core input maps; returns (in_maps, out_bias)."""
    x = np.asarray(x, np.float32)
    ac = np.asarray(all_controls, np.float32)
    W1 = np.asarray(W1, np.float32); b1 = np.asarray(b1, np.float32)
    W2 = np.asarray(W2, np.float32); b2 = np.asarray(b2, np.float32)
    W3 = np.asarray(W3, np.float32); b3 = np.asarray(b3, np.float32)
    ii = np.asarray(input_indices).astype(np.int64)
    ci = np.asarray(control_indices).astype(np.int64)

    # unfold (torch F.unfold ordering), pad=1, k=3, stride=1
    xp = np.pad(x, ((0, 0), (0, 0), (PAD, PAD), (PAD, PAD)))
    cols = [xp[:, :, i:i + HW, j:j + HW] for i in range(K) for j in range(K)]
    u = np.stack(cols, axis=2).reshape(B, IN_CH, K * K, L)
    u = u.transpose(0, 1, 3, 2).reshape(B, IN_CH, L, DEV, N_IN)

    # permuted W1 rows
    idx = np.concatenate([ii, ci], axis=-1)           # [DEV, 7]
    Wp = np.zeros((DEV, N_IN + N_CTRL, HID), np.float32)
    for d in range(DEV):
        for e in range(N_IN + N_CTRL):
            Wp[d, idx[d, e], :] = W1[e, :]
    A = Wp[:, :N_IN, :]                               # [DEV, 3, 90]
    C = Wp[:, N_IN:, :]                               # [DEV, 4, 90]
    cb = np.einsum('oidc,dch->oidh', ac, C) + b1      # [O, I, DEV, 90]
    b2f = b2 - W2.sum(axis=0)
    b3f = float((b3 - W3.sum(axis=0))[0])

    # lhs1: [16, O*NT, 91]; tile t covers combos 4t..4t+3, j = i*DEV+d
    lhs1 = np.zeros((16, OUT_CH * NT, M), np.float32)
    for o in range(OUT_CH):
        for t in range(NT):
            ot = o * NT + t
            for q in range(4):
                j = 4 * t + q
                i, d = j // DEV, j % DEV
                r = 4 * q
                lhs1[r:r + N_IN, ot, :HID] = A[d]
                lhs1[r + N_IN, ot, :HID] = cb[o, i, d] + 1.0
                lhs1[r + N_IN, ot, HID] = 1.0
    w2g = np.zeros((M, M), np.float32)
    w2g[:HID, :HID] = W2
    w2g[HID, :HID] = b2f + 1.0
    w2g[HID, HID] = 1.0
    w3g = np.zeros((M, 1), np.float32)
    w3g[:HID, 0] = W3[:, 0]                           # b3f folded on host

    in_maps = []
    for c in range(N_CORES):
        ub = u[c * B_LOC:(c + 1) * B_LOC]             # [2, I, L, DEV, 3]
        rhs1 = np.zeros((16, B_LOC * NT * X), np.float32)
        for b in range(B_LOC):
            for t in range(NT):
                base = (b * NT + t) * X
                for q in range(4):
                    j = 4 * t + q
                    i, d = j // DEV, j % DEV
                    r = 4 * q
                    cs = base + q * L
                    rhs1[r:r + N_IN, cs:cs + L] = ub[b, i, :, d, :].T
                    rhs1[r + N_IN, cs:cs + L] = 1.0
        in_maps.append({"rhs1": rhs1, "lhs1": lhs1, "w2g": w2g, "w3g": w3g})
    return in_maps, NJ * b3f


def kernel(x, all_controls, W1, b1, W2, b2, W3, b3,
           input_indices, control_indices):
    from concourse.bass_utils import run_bass_kernel_spmd

    nc = _get_program()
    in_maps, out_bias = _host_prep(x, all_controls, W1, b1, W2, b2, W3, b3,
                                   input_indices, control_indices)
    res = run_bass_kernel_spmd(nc, in_maps, list(range(N_CORES)))
    out = np.empty((B, OUT_CH, HW, HW), np.float32)
    for c in range(N_CORES):
        o_c = res.results[c]["out"].reshape(B_LOC, OUT_CH, 2, L)
        out[c * B_LOC:(c + 1) * B_LOC] = (
            o_c[:, :, 0, :] + o_c[:, :, 1, :] + out_bias
        ).reshape(B_LOC, OUT_CH, HW, HW)
    return out


# revision 9
# speedup vs baseline: 1.1513x; 1.1513x over previous
"""DNPUConv2d Trainium2 kernel (8 NeuronCores, batch-parallel).

Structure (per core; batch 16 -> 2 per core across 8 cores):

  zdata[b,i,l,d,:] = u @ A_d  is o-independent: computed once as 24
  [91,512] tiles (combo j=(i,d), columns (b,l)), then stored in SBUF as
  fp16 twice: E = exp(zdata) and zdata itself.

  Per (o, j) tile, using elu(z) = max(min(exp(z),1), z+1) - 1 with the
  -1 folded into the next layer's bias (ones-row trick), and
  exp(zdata + cb) = exp(zdata) * exp(cb):
    t  = (E * expcb[o,j]) min 1          VectorE tensor_scalar, fp16 4x
    g1 = (zdata + cbp1[o,j]) max t       VectorE stt, fp16 2x
    z2 = w2g^T @ g1                      PE (fp16)
    e2 = Exp(z2 - 1)                     ScalarE, PSUM -> fp16 SBUF
    even j: g2 = (e2 min 1) max z2       VectorE stt (PSUM operand, 1x)
    odd  j: r2 = Relu(z2 - 1) [ScalarE]; g2 = (e2 min 1) add r2 (2x)
    acc += w3g^T @ g2                    PE, 24 PSUM-accumulating matmuls
  The W3 dot rides after the (i,d) sum via linearity; b3f*24 added on
  host. PE order is software-pipelined: mm3(j-2) issues between mm2(j)
  tiles so the in-order PE queue never waits on the elementwise chain.
"""
import numpy as np

K = 3
PAD = 1
IN_CH = 8
OUT_CH = 16
DEV = 3
N_IN = 3
N_CTRL = 4
HID = 90
B = 16
HW = 16
L = HW * HW            # 256 output positions
N_CORES = 8
B_LOC = B // N_CORES   # 2 batches per core
M = HID + 1            # 91: hidden + ones row
NJ = IN_CH * DEV       # 24 (i,d) combos
X = B_LOC * L          # 512 = tile free dim (b, l)
PIPE = 2               # mm3 issue lag (tiles)

_COMPILED = {}


def _build_program():
    import concourse.bacc as bacc
    import concourse.tile as tile
    from concourse import mybir

    f32 = mybir.dt.float32
    f32r = mybir.dt.float32r
    f16 = mybir.dt.float16
    Exp = mybir.ActivationFunctionType.Exp
    Relu = mybir.ActivationFunctionType.Relu
    amin = mybir.AluOpType.min
    amax = mybir.AluOpType.max
    aadd = mybir.AluOpType.add
    amul = mybir.AluOpType.mult

    ZCOLS = NJ * X                # 12288 zdata columns

    nc = bacc.Bacc()
    # zdata rhs, one [3, IN_CH*X] block per device
    zr_d = nc.dram_tensor("zr", [DEV, N_IN, IN_CH * X], f32r,
                          kind="ExternalInput")
    lhsa_d = nc.dram_tensor("lhsa", [N_IN, DEV, M], f32r, kind="ExternalInput")
    w2g_d = nc.dram_tensor("w2g", [M, M], f16, kind="ExternalInput")
    w3g_d = nc.dram_tensor("w3g", [M, 1], f16, kind="ExternalInput")
    expcb_d = nc.dram_tensor("expcb", [M, OUT_CH * NJ], f32,
                             kind="ExternalInput")
    cbp1_d = nc.dram_tensor("cbp1", [M, OUT_CH * NJ], f32,
                            kind="ExternalInput")
    out_d = nc.dram_tensor("out", [OUT_CH, X], f32, kind="ExternalOutput")

    with tile.TileContext(nc) as tc:
        with (
            tc.tile_pool(name="singles", bufs=1) as singles,
            tc.tile_pool(name="work", bufs=4) as work,
            tc.tile_pool(name="g1p", bufs=6) as g1p,
            tc.tile_pool(name="g2p", bufs=PIPE + 2) as g2p,
            tc.tile_pool(name="outp", bufs=2) as outp,
            tc.tile_pool(name="psz", bufs=5, space="PSUM") as psz,
            tc.tile_pool(name="psacc", bufs=2, space="PSUM") as psacc,
        ):
            zr_sb = [singles.tile([N_IN, IN_CH * X], f32r, tag=f"zr{d}",
                                  name=f"zr_sb{d}") for d in range(DEV)]
            lhsa_sb = singles.tile([N_IN, DEV, M], f32r)
            w2g_sb = singles.tile([M, M], f16)
            w3g_sb = singles.tile([M, 1], f16)
            expcb_sb = singles.tile([M, OUT_CH * NJ], f32)
            cbp1_sb = singles.tile([M, OUT_CH * NJ], f32)
            E_sb = singles.tile([M, ZCOLS], f16)
            zd_sb = singles.tile([M, ZCOLS], f16)
            neg1 = singles.tile([128, 1], f32)
            ones = singles.tile([M, X], f16)
            nc.vector.memset(neg1, -1.0)
            nc.vector.memset(ones, 1.0)
            for d in range(DEV):
                nc.sync.dma_start(out=zr_sb[d], in_=zr_d[d])
            nc.sync.dma_start(out=lhsa_sb, in_=lhsa_d[:, :, :])
            nc.sync.dma_start(out=w2g_sb, in_=w2g_d[:, :])
            nc.sync.dma_start(out=w3g_sb, in_=w3g_d[:, :])
            nc.sync.dma_start(out=expcb_sb, in_=expcb_d[:, :])
            nc.sync.dma_start(out=cbp1_sb, in_=cbp1_d[:, :])

            # ---- precompute zdata and E = exp(zdata), both fp16 in SBUF ----
            for j in range(NJ):
                i, d = j // DEV, j % DEV
                zp = psz.tile([M, X], f32, tag="z", name="zp")
                nc.tensor.matmul(zp, lhsa_sb[:, d, :],
                                 zr_sb[d][:, i * X:(i + 1) * X],
                                 start=True, stop=True)
                nc.scalar.activation(E_sb[:, j * X:(j + 1) * X], zp, Exp)
                nc.vector.tensor_copy(zd_sb[:, j * X:(j + 1) * X], zp)

            # ---- main loop over output channels ----
            for o in range(OUT_CH):
                acc = psacc.tile([1, X], f32)
                g2s = {}
                n3 = 0

                def do_mm3(j3):
                    nonlocal n3
                    nc.tensor.matmul(acc, w3g_sb, g2s.pop(j3),
                                     start=(n3 == 0), stop=(n3 == NJ - 1))
                    n3 += 1

                for j in range(NJ):
                    oj = o * NJ + j
                    cs = j * X
                    t = work.tile([M, X], f16, tag="t")
                    nc.vector.scalar_tensor_tensor(
                        out=t, in0=E_sb[:, cs:cs + X],
                        scalar=expcb_sb[:, oj:oj + 1], in1=ones,
                        op0=amul, op1=amin)
                    g1 = g1p.tile([M, X], f16, tag="g1", name="g1")
                    nc.vector.scalar_tensor_tensor(
                        out=g1, in0=zd_sb[:, cs:cs + X],
                        scalar=cbp1_sb[:, oj:oj + 1], in1=t,
                        op0=aadd, op1=amax)
                    z2 = psz.tile([M, X], f32, tag="z", name="z2")
                    nc.tensor.matmul(z2, w2g_sb, g1, start=True, stop=True)
                    e2 = work.tile([M, X], f16, tag="e2")
                    nc.scalar.activation(e2, z2, Exp, bias=neg1[:M], scale=1.0)
                    g2 = g2p.tile([M, X], f16, tag="g2", name="g2")
                    if j % 2 == 0:
                        nc.vector.scalar_tensor_tensor(
                            out=g2, in0=e2, scalar=1.0, in1=z2,
                            op0=amin, op1=amax)
                    else:
                        r2 = work.tile([M, X], f16, tag="r2")
                        nc.scalar.activation(r2, z2, Relu, bias=neg1[:M],
                                             scale=1.0)
                        nc.vector.scalar_tensor_tensor(
                            out=g2, in0=e2, scalar=1.0, in1=r2,
                            op0=amin, op1=aadd)
                    g2s[j] = g2
                    if j >= PIPE:
                        do_mm3(j - PIPE)
                for j3 in range(NJ - PIPE, NJ):
                    do_mm3(j3)
                out_sb = outp.tile([1, X], f32, tag="osb", name="out_sb")
                nc.scalar.copy(out_sb, acc)
                nc.sync.dma_start(out=out_d[o:o + 1, :], in_=out_sb)

    nc.compile()
    return nc


def _get_program():
    if "nc" not in _COMPILED:
        _COMPILED["nc"] = _build_program()
    return _COMPILED["nc"]


def _host_prep(x, all_controls, W1, b1, W2, b2, W3, b3,
               input_indices, control_indices):
    """Build per-core input maps; returns (in_maps, out_bias)."""
    import ml_dtypes
    x = np.asarray(x, np.float32)
    ac = np.asarray(all_controls, np.float32)
    W1 = np.asarray(W1, np.float32); b1 = np.asarray(b1, np.float32)
    W2 = np.asarray(W2, np.float32); b2 = np.asarray(b2, np.float32)
    W3 = np.asarray(W3, np.float32); b3 = np.asarray(b3, np.float32)
    ii = np.asarray(input_indices).astype(np.int64)
    ci = np.asarray(control_indices).astype(np.int64)

    # unfold (torch F.unfold ordering), pad=1, k=3, stride=1
    xp = np.pad(x, ((0, 0), (0, 0), (PAD, PAD), (PAD, PAD)))
    cols = [xp[:, :, i:i + HW, j:j + HW] for i in range(K) for j in range(K)]
    u = np.stack(cols, axis=2).reshape(B, IN_CH, K * K, L)
    u = u.transpose(0, 1, 3, 2).reshape(B, IN_CH, L, DEV, N_IN)

    # permuted W1 rows
    idx = np.concatenate([ii, ci], axis=-1)           # [DEV, 7]
    Wp = np.zeros((DEV, N_IN + N_CTRL, HID), np.float32)
    for d in range(DEV):
        for e in range(N_IN + N_CTRL):
            Wp[d, idx[d, e], :] = W1[e, :]
    A = Wp[:, :N_IN, :]                               # [DEV, 3, 90]
    C = Wp[:, N_IN:, :]                               # [DEV, 4, 90]
    cb = np.einsum('oidc,dch->oidh', ac, C) + b1      # [O, I, DEV, 90]
    b2f = b2 - W2.sum(axis=0)
    b3f = float((b3 - W3.sum(axis=0))[0])

    lhsa = np.zeros((N_IN, DEV, M), np.float32)
    lhsa[:, :, :HID] = A.transpose(1, 0, 2)
    w2g = np.zeros((M, M), np.float32)
    w2g[:HID, :HID] = W2
    w2g[HID, :HID] = b2f + 1.0
    w2g[HID, HID] = 1.0
    w3g = np.zeros((M, 1), np.float32)
    w3g[:HID, 0] = W3[:, 0]                           # b3f folded on host

    # per-(o, j) per-partition vectors: expcb = exp(cb), cbp1 = cb + 1
    expcb = np.ones((M, OUT_CH * NJ), np.float32)
    cbp1 = np.ones((M, OUT_CH * NJ), np.float32)
    for o in range(OUT_CH):
        for j in range(NJ):
            i, d = j // DEV, j % DEV
            oj = o * NJ + j
            expcb[:HID, oj] = np.exp(cb[o, i, d])
            cbp1[:HID, oj] = cb[o, i, d] + 1.0

    w2g16 = w2g.astype(np.float16)
    w3g16 = w3g.astype(np.float16)

    in_maps = []
    for c in range(N_CORES):
        ub = u[c * B_LOC:(c + 1) * B_LOC]             # [2, I, L, DEV, 3]
        # zr[d]: [3, IN_CH * X] with col = i*X + b*L + l
        zr = np.empty((DEV, N_IN, IN_CH * X), np.float32)
        for d in range(DEV):
            # ub[b,i,l,d,n] -> [n, i, b, l]
            zr[d] = ub[:, :, :, d, :].transpose(3, 1, 0, 2).reshape(
                N_IN, IN_CH * X)
        in_maps.append({"zr": zr, "lhsa": lhsa, "w2g": w2g16, "w3g": w3g16,
                        "expcb": expcb, "cbp1": cbp1})
    return in_maps, NJ * b3f


def kernel(x, all_controls, W1, b1, W2, b2, W3, b3,
           input_indices, control_indices):
    from concourse.bass_utils import run_bass_kernel_spmd

    nc = _get_program()
    in_maps, out_bias = _host_prep(x, all_controls, W1, b1, W2, b2, W3, b3,
                                   input_indices, control_indices)
    res = run_bass_kernel_spmd(nc, in_maps, list(range(N_CORES)))
    out = np.empty((B, OUT_CH, HW, HW), np.float32)
    for c in range(N_CORES):
        o_c = res.results[c]["out"].reshape(OUT_CH, B_LOC, L)
        out[c * B_LOC:(c + 1) * B_LOC] = (
            o_c.transpose(1, 0, 2) + out_bias).reshape(B_LOC, OUT_CH, HW, HW)
    return out
